# revision 39
# baseline (speedup 1.0000x reference)
"""Trainium2 Bass kernel for nn_LINKX (GNN message passing + dense head).

Contract: kernel(**inputs) takes FULL unsharded inputs (numpy arrays keyed as
in setup_inputs()) and returns the FULL [N, OUT_C] float32 output.

Strategy (8 cores, graph-parallel by destination node):
  - Fold the dense prologue algebraically on host:
        h  = leaky(A @ T + x @ NW2 + c)          T   = edge_lin_weight @ (I+cat1)
        g  = leaky(h @ W0.T + b0)                NW2 = node_w @ (I+cat2)
        y  = leaky(g @ W1.T + b1)
    where A is the sparse [N,N] matrix with A[dst,src] += edge_weight, and
    W0/W1 are the host-computed modulated+row-normalized synthesis weights.
  - Destination nodes are sorted by degree and dealt into 8 cores x 196
    blocks of 64, so each block holds similar-degree nodes.  Edges use a
    FIXED slot->dst mapping (dst_local = slot % 64, two slots per dst per
    128-slot column), so the segment-sum reduction is a matmul against one
    small resident constant selector [I64; I64]/64 - no selector stream.
    Host resolves the gather: messages 64*w_e*T[src_e] are packed fp8(e4m3)
    into the slot layout (ncols per block = ceil(max block degree / 2),
    shared across cores).  The device streams messages sequentially and runs
        acc[h, d] = sum_col  msg_col[slot, h]^T @ Sconst[slot, d]   (fp8)
                  + NW2^T x^T                                       (fp16)
    per 8-block superblock into one PSUM bank, then the fp16 dense chain
    (bias-add + leaky on the Vector engine) produces y [64, 512] fp16 per
    superblock.  Host un-permutes rows of y at the end.
"""

import math
import numpy as np

import concourse.bacc as bacc
import concourse.mybir as mybir
import concourse.tile as tile

F32 = mybir.dt.float32
F16 = mybir.dt.float16
F8 = mybir.dt.float8e4
SLOPE = 0.01
RANK = 10

# -------------------- problem constants (hardcoded) --------------------
N_NODES = 100000
N_EDGES = 1600000
IN_C = 128
H = 128
OUT_C = 64
N_CORES = 8

DB = 64                          # dst block width
PN_PAD = 12544                   # padded dsts per core (196 * 64)
NBLK = PN_PAD // DB              # 196 block levels
SBLK = 8                         # blocks per superblock (512 dst, 1 PSUM bank)
NSB = math.ceil(NBLK / SBLK)     # 25 superblocks (last one partial: 4 blocks)
NPAD = N_CORES * PN_PAD - N_NODES  # virtual zero-degree dsts
MSG_SCALE = 64.0                 # msgs stored *64, Sconst entries 1/64 (exact)


def host_weights(inputs):
    """Fold the dense algebra on host (float64 for the tiny mats)."""
    f8 = np.float64
    I = np.eye(H, dtype=f8)
    cat1 = np.asarray(inputs["cat1_w"], f8)
    cat2 = np.asarray(inputs["cat2_w"], f8)
    node_w = np.asarray(inputs["node_w"], f8)
    C1 = I + cat1
    C2 = I + cat2
    NW2 = node_w @ C2
    c = (np.asarray(inputs["edge_lin_bias"], f8) @ C1
         + np.asarray(inputs["cat1_b"], f8)
         + np.asarray(inputs["node_b"], f8) @ C2
         + np.asarray(inputs["cat2_b"], f8))
    wvec = np.asarray(inputs["w"], f8)

    def synth(aff_w, aff_b, weight):
        c_out, c_in = weight.shape
        styles = wvec[0 if c_out == H else 1] @ np.asarray(aff_w, f8) \
            + np.asarray(aff_b, f8)
        left = styles[: c_out * RANK].reshape(c_out, RANK)
        right = styles[c_out * RANK:].reshape(RANK, c_in)
        mod = (left @ right) / np.sqrt(np.float64(RANK))
        W = np.asarray(weight, f8) * (mod + 1.0)
        W = W / (np.linalg.norm(W, axis=1, keepdims=True) + 1e-8)
        return W

    W0 = synth(inputs["syn0_aff_w"], inputs["syn0_aff_b"],
               np.asarray(inputs["syn0_weight"], f8))
    W1 = synth(inputs["syn1_aff_w"], inputs["syn1_aff_b"],
               np.asarray(inputs["syn1_weight"], f8))

    T = np.asarray(inputs["edge_lin_weight"], np.float32) @ C1.astype(np.float32)

    return dict(
        T=np.ascontiguousarray(T, np.float32),
        NW2=np.ascontiguousarray(NW2, np.float16),
        cvec=np.ascontiguousarray(c.reshape(H, 1), np.float32),
        W0T=np.ascontiguousarray(W0.T, np.float16),
        W1T=np.ascontiguousarray(W1.T, np.float16),
        b0=np.ascontiguousarray(np.asarray(inputs["syn0_bias"], f8).reshape(H, 1),
                                np.float32),
        b1=np.ascontiguousarray(np.asarray(inputs["syn1_bias"], f8).reshape(OUT_C, 1),
                                np.float32),
    )


def plan(dst):
    """Degree-sorted dst assignment + per-block-level column counts.

    Node n sits at rank r = node_r[n]: block level b = r//512,
    core = (r%512)//64, pos = r%64.  Ranks [0, NPAD) are virtual padding."""
    deg = np.bincount(dst, minlength=N_NODES)
    order = np.argsort(deg, kind="stable")
    node_r = np.empty(N_NODES, np.int64)
    node_r[order] = NPAD + np.arange(N_NODES)
    sdeg = np.zeros(N_CORES * PN_PAD, np.int64)
    sdeg[NPAD:] = deg[order]
    mx = sdeg.reshape(NBLK, N_CORES * DB).max(axis=1)
    ncols = np.maximum((mx + 1) // 2, 1).astype(np.int64)
    col_off = np.zeros(NBLK + 1, np.int64)
    np.cumsum(ncols, out=col_off[1:])
    return ncols, col_off, node_r


def host_prep_core(k, src, dst, w, T, node_r, col_off):
    """Pack the fp8 message stream for core k (fixed slot%64 = dst_local)."""
    f8np = mybir.dt.np(F8)
    totcols = int(col_off[-1])
    r = node_r[dst]
    core = (r % 512) // DB
    m = core == k
    r_k = r[m]
    s_k = src[m]
    w_k = w[m].astype(np.float32)
    order = np.argsort(r_k, kind="stable")
    r_s = r_k[order]
    grp_change = np.empty(len(r_s), dtype=bool)
    if len(r_s):
        grp_change[0] = True
        grp_change[1:] = r_s[1:] != r_s[:-1]
    grp_start = np.maximum.accumulate(
        np.where(grp_change, np.arange(len(r_s)), 0))
    j = np.arange(len(r_s)) - grp_start
    b_s = r_s // 512
    pos_s = r_s % DB
    slot = (col_off[b_s] + j // 2) * 128 + pos_s + DB * (j & 1)

    tot_slots = totcols * 128
    msg = np.zeros((tot_slots, H), f8np)
    vals = (MSG_SCALE * w_k[order])[:, None] * T[s_k[order]]
    msg[slot] = vals.astype(f8np)
    return np.ascontiguousarray(
        msg.reshape(totcols, 128, H).transpose(1, 0, 2))


def make_sconst():
    f8np = mybir.dt.np(F8)
    s = np.zeros((128, DB), np.float32)
    s[np.arange(128), np.arange(128) % DB] = 1.0 / MSG_SCALE
    return np.ascontiguousarray(s.astype(f8np))


def build_kernel_body(tc, ncols, col_off, outs, ins):
    nc = tc.nc
    msgs, xt = ins["msgs"], ins["xt"]
    wblob, bblob, sconst = ins["wblob"], ins["bblob"], ins["sconst"]
    yout = outs["y"]

    ADD = mybir.AluOpType.add
    MULT = mybir.AluOpType.mult
    MAX = mybir.AluOpType.max
    LRELU = mybir.ActivationFunctionType.Lrelu

    with (
        tc.tile_pool(name="const", bufs=1) as cp,
        tc.tile_pool(name="mpool", bufs=6) as mp,
        tc.tile_pool(name="hpool", bufs=2) as hp,
        tc.tile_pool(name="gpool", bufs=2) as gp,
        tc.tile_pool(name="ypool", bufs=2) as yp,
        tc.tile_pool(name="pacc", bufs=3, space="PSUM") as paccp,
        tc.tile_pool(name="p1", bufs=3, space="PSUM") as p1p,
        tc.tile_pool(name="p2", bufs=2, space="PSUM") as p2p,
    ):
        sconst_sb = cp.tile([128, DB], F8)
        nc.scalar.dma_start(sconst_sb[:], sconst[:])
        wblob_sb = cp.tile([H, 2 * H + OUT_C + 3], F16)
        nc.scalar.dma_start(wblob_sb[:], wblob[:])
        bblob_sb = cp.tile([H, 3], F32)
        nc.scalar.dma_start(bblob_sb[:], bblob[:])
        nw2_sb = wblob_sb[:, 0:H]
        w0t_sb = wblob_sb[:, H: 2 * H]
        w1t_sb = wblob_sb[:, 2 * H: 2 * H + OUT_C]
        cvec_sb = bblob_sb[:, 0:1]
        b0_sb = bblob_sb[:, 1:2]
        b1_sb = bblob_sb[0:OUT_C, 2:3]
        # resident x chunks (5 groups of 5 superblocks) + y group tiles
        GRP = 5
        gw = GRP * SBLK * DB
        gd0 = [min(gi * gw, PN_PAD) for gi in range(6)]
        x_grp = [cp.tile([H, gd0[gi + 1] - gd0[gi]], F16, name=f"x_grp{gi}")
                 for gi in range(5)]
        nc.scalar.dma_start(x_grp[0][:], xt[:, gd0[0]:gd0[1]])
        y_grp = [cp.tile([OUT_C, gd0[gi + 1] - gd0[gi]], F16,
                         name=f"y_grp{gi}") for gi in range(5)]

        max_sb_cols = max(
            int(col_off[min(si * SBLK + SBLK, NBLK)] - col_off[si * SBLK])
            for si in range(NSB)
        )

        # Software pipeline: phase A(si) streams + reduces superblock si into
        # its PSUM bank; phase B(si) runs the dense chain on the previous
        # superblock while A(si+1)'s matmuls keep the PE busy.
        state = {}

        def phase_a(si):
            if si >= 2 and si % GRP == 2:
                gi = si // GRP + 1
                if gi < 5:
                    nc.scalar.dma_start(x_grp[gi][:], xt[:, gd0[gi]:gd0[gi + 1]])
            blocks = list(range(si * SBLK, min(si * SBLK + SBLK, NBLK)))
            sbn = len(blocks)
            c0 = int(col_off[blocks[0]])
            c1 = int(col_off[blocks[-1] + 1])
            ncol_sb = c1 - c0

            cmid = ncol_sb // 2
            half = (max_sb_cols + 1) // 2
            p0 = min(8, cmid) if si == 0 else 0
            if p0:
                msg_p = mp.tile([128, 8, H], F8, tag="mp", name="msg_p")
                nc.sync.dma_start(msg_p[:, :p0, :], msgs[:, c0:c0 + p0, :])
            msg_a = mp.tile([128, half, H], F8, tag="ma", name="msg_a")
            nc.sync.dma_start(msg_a[:, : cmid - p0, :],
                              msgs[:, c0 + p0:c0 + cmid, :])
            msg_b = mp.tile([128, half, H], F8, tag="mb", name="msg_b")
            nc.sync.dma_start(msg_b[:, : ncol_sb - cmid, :],
                              msgs[:, c0 + cmid:c1, :])


            acc = paccp.tile([H, SBLK, DB], F32, tag="acc")
            # Round-robin across blocks so consecutive matmuls hit different
            # PSUM windows.  First matmul's start=True zeroes the whole 2KB
            # PSUM bank; the x-part runs last (stop=True) so superblock 0
            # needn't wait for the x load.
            cols = []
            maxp = max(int(ncols[b]) for b in blocks)
            for p in range(maxp):
                for bi, b in enumerate(blocks):
                    if p < int(ncols[b]):
                        c = int(col_off[b]) - c0
                        cols.append((bi, c + p))
            cols.sort(key=lambda t: (t[1] >= p0) + (t[1] >= cmid))
            for mm, (bi, c) in enumerate(cols):
                if c < p0:
                    mt = msg_p[:, c, :]
                elif c < cmid:
                    mt = msg_a[:, c - p0, :]
                else:
                    mt = msg_b[:, c - cmid, :]
                nc.tensor.matmul(
                    acc[:, bi, :],
                    lhsT=mt,
                    rhs=sconst_sb[:],
                    start=(mm == 0), stop=False,
                )
            xg = x_grp[si // GRP]
            xo = (si % GRP) * SBLK * DB
            nc.tensor.matmul(acc[:, :sbn, :], lhsT=nw2_sb,
                             rhs=xg[:, xo: xo + sbn * DB], start=False,
                             stop=True)
            state[si] = (blocks, sbn, acc)

        def leaky_dve(pool, psum_ap, bias_ap, p, f, tag):
            # t = psum + bias (per-partition);  out = max(0.01*t, t)
            t_t = pool.tile([p, f], F16, tag=tag + "t", name=tag + "t")
            nc.vector.tensor_scalar(t_t[:, :f], psum_ap, bias_ap, None, ADD)
            o_t = pool.tile([p, f], F16, tag=tag + "o", name=tag + "o")
            nc.vector.scalar_tensor_tensor(o_t[:, :f], t_t[:, :f], SLOPE,
                                           t_t[:, :f], MULT, MAX)
            return o_t

        def leaky_act(pool, psum_ap, bias_ap, p, f, tag):
            o_t = pool.tile([p, f], F16, tag=tag + "o", name=tag + "o")
            nc.scalar.activation(o_t[:, :f], psum_ap, LRELU, bias=bias_ap,
                                 scale=1.0, alpha=SLOPE)
            return o_t

        def phase_b(si):
            blocks, sbn, acc = state.pop(si)
            h_t = leaky_act(hp, acc[:, :sbn, :], cvec_sb, H, sbn * DB, "h")
            ps1 = p1p.tile([H, SBLK * DB], F32, tag="p1")
            nc.tensor.matmul(ps1[:, : sbn * DB], lhsT=w0t_sb,
                             rhs=h_t[:, : sbn * DB], start=True, stop=True)
            g_t = leaky_dve(gp, ps1[:, : sbn * DB], b0_sb, H, sbn * DB, "g")
            ps2 = p2p.tile([OUT_C, SBLK * DB], F32, tag="p2")
            nc.tensor.matmul(ps2[:, : sbn * DB], lhsT=w1t_sb,
                             rhs=g_t[:, : sbn * DB], start=True, stop=True)
            if si // GRP == 4:
                y_t = yp.tile([OUT_C, SBLK * DB], F16, tag="y", name="y_t")
                nc.scalar.activation(y_t[:, : sbn * DB], ps2[:, : sbn * DB],
                                     LRELU, bias=b1_sb, scale=1.0, alpha=SLOPE)
                d0 = blocks[0] * DB
                nc.scalar.dma_start(yout[:, d0: d0 + sbn * DB],
                                    y_t[:, : sbn * DB])
            else:
                yg = y_grp[si // GRP]
                yo = (si % GRP) * SBLK * DB
                nc.scalar.activation(yg[:, yo: yo + sbn * DB],
                                     ps2[:, : sbn * DB], LRELU, bias=b1_sb,
                                     scale=1.0, alpha=SLOPE)
                if si % GRP == GRP - 1:
                    gi = si // GRP
                    nc.scalar.dma_start(yout[:, gd0[gi]:gd0[gi + 1]], yg[:])

        for si in range(NSB + 1):
            if si < NSB:
                phase_a(si)
            if si >= 1:
                phase_b(si - 1)


def declare_tensors(nc, totcols):
    d = nc.dram_tensor
    ins = dict(
        msgs=d("msgs", [128, totcols, H], F8, kind="ExternalInput")[:, :, :],
        xt=d("xt", [H, PN_PAD], F16, kind="ExternalInput")[:, :],
        wblob=d("wblob", [H, 2 * H + OUT_C + 3], F16,
                kind="ExternalInput")[:, :],
        bblob=d("bblob", [H, 3], F32, kind="ExternalInput")[:, :],
        sconst=d("sconst", [128, DB], F8, kind="ExternalInput")[:, :],
    )
    outs = dict(y=d("y", [OUT_C, PN_PAD], F16, kind="ExternalOutput")[:, :])
    return ins, outs


def build_nc(ncols, col_off):
    nc = bacc.Bacc("TRN2", target_bir_lowering=False, debug=False,
                   num_devices=N_CORES)
    ins, outs = declare_tensors(nc, int(col_off[-1]))
    with tile.TileContext(nc) as tc:
        build_kernel_body(tc, ncols, col_off, outs, ins)
    nc.compile()
    return nc


def make_in_maps(inputs):
    hw = host_weights(inputs)
    edge_index = np.asarray(inputs["edge_index"])
    src = edge_index[0].astype(np.int64)
    dst = edge_index[1].astype(np.int64)
    w = np.asarray(inputs["edge_weight"], np.float32)
    x = np.asarray(inputs["x"], np.float32)

    ncols, col_off, node_r = plan(dst)

    bias = np.zeros((H, 3), np.float16)
    wblob = np.concatenate([hw["NW2"], hw["W0T"], hw["W1T"], bias], axis=1)
    bblob = np.zeros((H, 3), np.float32)
    bblob[:, 0:1] = hw["cvec"]
    bblob[:, 1:2] = hw["b0"]
    bblob[:OUT_C, 2:3] = hw["b1"]
    sconst = make_sconst()

    # node n -> (core, column b*64 + pos) under the degree-sorted permutation
    b_n = node_r // 512
    core_n = (node_r % 512) // DB
    pos_n = node_r % DB
    colpos = b_n * DB + pos_n

    in_maps = []
    for k in range(N_CORES):
        msg = host_prep_core(k, src, dst, w, hw["T"], node_r, col_off)
        xtk = np.zeros((H, PN_PAD), np.float16)
        mk = core_n == k
        xtk[:, colpos[mk]] = x[mk].T
        in_maps.append(dict(
            msgs=msg, xt=np.ascontiguousarray(xtk),
            wblob=np.ascontiguousarray(wblob), bblob=bblob, sconst=sconst,
        ))
    return in_maps, ncols, col_off, (core_n, colpos)


_CACHE = {}
LAST_RESULTS = None


def kernel(**inputs) -> np.ndarray:
    global LAST_RESULTS
    import os
    from concourse.bass_utils import run_bass_kernel_spmd

    in_maps, ncols, col_off, (core_n, colpos) = make_in_maps(inputs)

    key = ("nc", tuple(int(v) for v in ncols))
    if key not in _CACHE:
        _CACHE[key] = build_nc(ncols, col_off)
    nc = _CACHE[key]

    trace = bool(int(os.environ.get("LINKX_TRACE", "0")))
    res = run_bass_kernel_spmd(nc, in_maps, core_ids=list(range(N_CORES)),
                               trace=trace)
    LAST_RESULTS = res
    out = np.empty((N_NODES, OUT_C), np.float32)
    for k in range(N_CORES):
        yk = np.asarray(res.results[k]["y"], np.float32)
        mk = core_n == k
        out[mk] = yk[:, colpos[mk]].T
    return out


# revision 40
# speedup vs baseline: 1.1084x; 1.1084x over previous
"""Trainium2 Bass kernel for nn_LINKX (GNN message passing + dense head).

Contract: kernel(**inputs) takes FULL unsharded inputs (numpy arrays keyed as
in setup_inputs()) and returns the FULL [N, OUT_C] float32 output.

Strategy (8 cores, graph-parallel by destination node):
  - Fold the dense prologue algebraically on host:
        h  = leaky(A @ T + x @ NW2 + c)          T   = edge_lin_weight @ (I+cat1)
        g  = leaky(h @ W0.T + b0)                NW2 = node_w @ (I+cat2)
        y  = leaky(g @ W1.T + b1)
    where A is the sparse [N,N] matrix with A[dst,src] += edge_weight, and
    W0/W1 are the host-computed modulated+row-normalized synthesis weights.
  - Destination nodes are sorted by degree and dealt into 8 cores x 196
    blocks of 64, so each block holds similar-degree nodes.  Edges use a
    FIXED slot->dst mapping (dst_local = slot % 64, two slots per dst per
    128-slot column), so the segment-sum reduction is a matmul against one
    small resident constant selector [I64; I64]/64 - no selector stream.
    Host resolves the gather: messages 64*w_e*T[src_e] are packed fp8(e4m3)
    into the slot layout (ncols per block = ceil(max block degree / 2),
    shared across cores).  The device streams messages sequentially and runs
        acc[h, d] = sum_col  msg_col[slot, h]^T @ Sconst[slot, d]   (fp8)
                  + NW2^T x^T                                       (fp16)
    per 8-block superblock into one PSUM bank, then the fp16 dense chain
    (bias-add + leaky on the Vector engine) produces y [64, 512] fp16 per
    superblock.  Host un-permutes rows of y at the end.
"""

import math
import numpy as np

import concourse.bacc as bacc
import concourse.mybir as mybir
import concourse.tile as tile

F32 = mybir.dt.float32
F16 = mybir.dt.float16
F8 = mybir.dt.float8e4
SLOPE = 0.01
RANK = 10

# -------------------- problem constants (hardcoded) --------------------
N_NODES = 100000
N_EDGES = 1600000
IN_C = 128
H = 128
OUT_C = 64
N_CORES = 8

DB = 64                          # dst block width
PN_PAD = 12544                   # padded dsts per core (196 * 64)
NBLK = PN_PAD // DB              # 196 block levels
SBLK = 8                         # blocks per superblock (512 dst, 1 PSUM bank)
NSB = math.ceil(NBLK / SBLK)     # 25 superblocks (last one partial: 4 blocks)
NPAD = N_CORES * PN_PAD - N_NODES  # virtual zero-degree dsts
MSG_SCALE = 64.0                 # msgs stored *64, Sconst entries 1/64 (exact)


def host_weights(inputs):
    """Fold the dense algebra on host (float64 for the tiny mats)."""
    f8 = np.float64
    I = np.eye(H, dtype=f8)
    cat1 = np.asarray(inputs["cat1_w"], f8)
    cat2 = np.asarray(inputs["cat2_w"], f8)
    node_w = np.asarray(inputs["node_w"], f8)
    C1 = I + cat1
    C2 = I + cat2
    NW2 = node_w @ C2
    c = (np.asarray(inputs["edge_lin_bias"], f8) @ C1
         + np.asarray(inputs["cat1_b"], f8)
         + np.asarray(inputs["node_b"], f8) @ C2
         + np.asarray(inputs["cat2_b"], f8))
    wvec = np.asarray(inputs["w"], f8)

    def synth(aff_w, aff_b, weight):
        c_out, c_in = weight.shape
        styles = wvec[0 if c_out == H else 1] @ np.asarray(aff_w, f8) \
            + np.asarray(aff_b, f8)
        left = styles[: c_out * RANK].reshape(c_out, RANK)
        right = styles[c_out * RANK:].reshape(RANK, c_in)
        mod = (left @ right) / np.sqrt(np.float64(RANK))
        W = np.asarray(weight, f8) * (mod + 1.0)
        W = W / (np.linalg.norm(W, axis=1, keepdims=True) + 1e-8)
        return W

    W0 = synth(inputs["syn0_aff_w"], inputs["syn0_aff_b"],
               np.asarray(inputs["syn0_weight"], f8))
    W1 = synth(inputs["syn1_aff_w"], inputs["syn1_aff_b"],
               np.asarray(inputs["syn1_weight"], f8))

    T = np.asarray(inputs["edge_lin_weight"], np.float32) @ C1.astype(np.float32)

    return dict(
        T=np.ascontiguousarray(T, np.float32),
        NW2=np.ascontiguousarray(NW2, np.float16),
        cvec=np.ascontiguousarray(c.reshape(H, 1), np.float32),
        W0T=np.ascontiguousarray(W0.T, np.float16),
        W1T=np.ascontiguousarray(W1.T, np.float16),
        b0=np.ascontiguousarray(np.asarray(inputs["syn0_bias"], f8).reshape(H, 1),
                                np.float32),
        b1=np.ascontiguousarray(np.asarray(inputs["syn1_bias"], f8).reshape(OUT_C, 1),
                                np.float32),
    )


def plan(dst):
    """Degree-sorted dst assignment + per-block-level column counts.

    Node n sits at rank r = node_r[n]: block level b = r//512,
    core = (r%512)//64, pos = r%64.  Ranks [0, NPAD) are virtual padding."""
    deg = np.bincount(dst, minlength=N_NODES)
    order = np.argsort(deg, kind="stable")
    node_r = np.empty(N_NODES, np.int64)
    node_r[order] = NPAD + np.arange(N_NODES)
    sdeg = np.zeros(N_CORES * PN_PAD, np.int64)
    sdeg[NPAD:] = deg[order]
    mx = sdeg.reshape(NBLK, N_CORES * DB).max(axis=1)
    ncols = np.maximum((mx + 1) // 2, 1).astype(np.int64)
    col_off = np.zeros(NBLK + 1, np.int64)
    np.cumsum(ncols, out=col_off[1:])
    return ncols, col_off, node_r


def host_prep_core(k, src, dst, w, T, node_r, col_off):
    """Pack the fp8 message stream for core k (fixed slot%64 = dst_local)."""
    f8np = mybir.dt.np(F8)
    totcols = int(col_off[-1])
    r = node_r[dst]
    core = (r % 512) // DB
    m = core == k
    r_k = r[m]
    s_k = src[m]
    w_k = w[m].astype(np.float32)
    order = np.argsort(r_k, kind="stable")
    r_s = r_k[order]
    grp_change = np.empty(len(r_s), dtype=bool)
    if len(r_s):
        grp_change[0] = True
        grp_change[1:] = r_s[1:] != r_s[:-1]
    grp_start = np.maximum.accumulate(
        np.where(grp_change, np.arange(len(r_s)), 0))
    j = np.arange(len(r_s)) - grp_start
    b_s = r_s // 512
    pos_s = r_s % DB
    slot = (col_off[b_s] + j // 2) * 128 + pos_s + DB * (j & 1)

    tot_slots = totcols * 128
    msg = np.zeros((tot_slots, H), f8np)
    vals = (MSG_SCALE * w_k[order])[:, None] * T[s_k[order]]
    msg[slot] = vals.astype(f8np)
    return np.ascontiguousarray(
        msg.reshape(totcols, 128, H).transpose(1, 0, 2))


def make_sconst():
    f8np = mybir.dt.np(F8)
    s = np.zeros((128, DB), np.float32)
    s[np.arange(128), np.arange(128) % DB] = 1.0 / MSG_SCALE
    return np.ascontiguousarray(s.astype(f8np))


def build_kernel_body(tc, ncols, col_off, outs, ins):
    nc = tc.nc
    msgs, xt = ins["msgs"], ins["xt"]
    wblob, bblob, sconst = ins["wblob"], ins["bblob"], ins["sconst"]
    yout = outs["y"]

    ADD = mybir.AluOpType.add
    MULT = mybir.AluOpType.mult
    MAX = mybir.AluOpType.max
    LRELU = mybir.ActivationFunctionType.Lrelu

    with (
        tc.tile_pool(name="const", bufs=1) as cp,
        tc.tile_pool(name="mpool", bufs=6) as mp,
        tc.tile_pool(name="hpool", bufs=2) as hp,
        tc.tile_pool(name="gpool", bufs=2) as gp,
        tc.tile_pool(name="pacc", bufs=3, space="PSUM") as paccp,
        tc.tile_pool(name="p1", bufs=3, space="PSUM") as p1p,
        tc.tile_pool(name="p2", bufs=2, space="PSUM") as p2p,
    ):
        sconst_sb = cp.tile([128, DB], F8)
        nc.scalar.dma_start(sconst_sb[:], sconst[:])
        wblob_sb = cp.tile([H, 2 * H + OUT_C + 3], F16)
        nc.scalar.dma_start(wblob_sb[:], wblob[:])
        bblob_sb = cp.tile([H, 3], F32)
        nc.scalar.dma_start(bblob_sb[:], bblob[:])
        nw2_sb = wblob_sb[:, 0:H]
        w0t_sb = wblob_sb[:, H: 2 * H]
        w1t_sb = wblob_sb[:, 2 * H: 2 * H + OUT_C]
        cvec_sb = bblob_sb[:, 0:1]
        b0_sb = bblob_sb[:, 1:2]
        b1_sb = bblob_sb[0:OUT_C, 2:3]
        # resident x chunks (5 groups of 5 superblocks) + y group tiles
        GRP = 5
        gw = GRP * SBLK * DB
        gd0 = [min(gi * gw, PN_PAD) for gi in range(6)]
        x_grp = [cp.tile([H, gd0[gi + 1] - gd0[gi]], F16, name=f"x_grp{gi}")
                 for gi in range(5)]
        nc.scalar.dma_start(x_grp[0][:], xt[:, gd0[0]:gd0[1]])
        y_grp = [cp.tile([OUT_C, gd0[gi + 1] - gd0[gi]], F16,
                         name=f"y_grp{gi}") for gi in range(5)]

        max_sb_cols = max(
            int(col_off[min(si * SBLK + SBLK, NBLK)] - col_off[si * SBLK])
            for si in range(NSB)
        )

        # Software pipeline: phase A(si) streams + reduces superblock si into
        # its PSUM bank; phase B(si) runs the dense chain on the previous
        # superblock while A(si+1)'s matmuls keep the PE busy.
        state = {}

        def phase_a(si):
            if si >= 2 and si % GRP == 2:
                gi = si // GRP + 1
                if gi < 5:
                    nc.scalar.dma_start(x_grp[gi][:], xt[:, gd0[gi]:gd0[gi + 1]])
            blocks = list(range(si * SBLK, min(si * SBLK + SBLK, NBLK)))
            sbn = len(blocks)
            c0 = int(col_off[blocks[0]])
            c1 = int(col_off[blocks[-1] + 1])
            ncol_sb = c1 - c0

            cmid = ncol_sb // 2
            half = (max_sb_cols + 1) // 2
            msg_a = mp.tile([128, half, H], F8, tag="ma", name="msg_a")
            nc.sync.dma_start(msg_a[:, :cmid, :], msgs[:, c0:c0 + cmid, :])
            msg_b = mp.tile([128, half, H], F8, tag="mb", name="msg_b")
            nc.sync.dma_start(msg_b[:, : ncol_sb - cmid, :],
                              msgs[:, c0 + cmid:c1, :])


            acc = paccp.tile([H, SBLK, DB], F32, tag="acc")
            # Round-robin across blocks so consecutive matmuls hit different
            # PSUM windows.  First matmul's start=True zeroes the whole 2KB
            # PSUM bank; the x-part runs last (stop=True) so superblock 0
            # needn't wait for the x load.
            cols = []
            maxp = max(int(ncols[b]) for b in blocks)
            for p in range(maxp):
                for bi, b in enumerate(blocks):
                    if p < int(ncols[b]):
                        c = int(col_off[b]) - c0
                        cols.append((bi, c + p))
            cols.sort(key=lambda t: t[1] >= cmid)
            for mm, (bi, c) in enumerate(cols):
                mt = msg_a[:, c, :] if c < cmid else msg_b[:, c - cmid, :]
                nc.tensor.matmul(
                    acc[:, bi, :],
                    lhsT=mt,
                    rhs=sconst_sb[:],
                    start=(mm == 0), stop=False,
                )
            xg = x_grp[si // GRP]
            xo = (si % GRP) * SBLK * DB
            nc.tensor.matmul(acc[:, :sbn, :], lhsT=nw2_sb,
                             rhs=xg[:, xo: xo + sbn * DB], start=False,
                             stop=True)
            state[si] = (blocks, sbn, acc)

        def leaky_dve(pool, psum_ap, bias_ap, p, f, tag):
            # t = psum + bias (per-partition);  out = max(0.01*t, t)
            t_t = pool.tile([p, f], F16, tag=tag + "t", name=tag + "t")
            nc.vector.tensor_scalar(t_t[:, :f], psum_ap, bias_ap, None, ADD)
            o_t = pool.tile([p, f], F16, tag=tag + "o", name=tag + "o")
            nc.vector.scalar_tensor_tensor(o_t[:, :f], t_t[:, :f], SLOPE,
                                           t_t[:, :f], MULT, MAX)
            return o_t

        def leaky_act(pool, psum_ap, bias_ap, p, f, tag):
            o_t = pool.tile([p, f], F16, tag=tag + "o", name=tag + "o")
            nc.scalar.activation(o_t[:, :f], psum_ap, LRELU, bias=bias_ap,
                                 scale=1.0, alpha=SLOPE)
            return o_t

        def phase_b(si):
            blocks, sbn, acc = state.pop(si)
            h_t = leaky_act(hp, acc[:, :sbn, :], cvec_sb, H, sbn * DB, "h")
            ps1 = p1p.tile([H, SBLK * DB], F32, tag="p1")
            nc.tensor.matmul(ps1[:, : sbn * DB], lhsT=w0t_sb,
                             rhs=h_t[:, : sbn * DB], start=True, stop=True)
            g_t = leaky_dve(gp, ps1[:, : sbn * DB], b0_sb, H, sbn * DB, "g")
            ps2 = p2p.tile([OUT_C, SBLK * DB], F32, tag="p2")
            nc.tensor.matmul(ps2[:, : sbn * DB], lhsT=w1t_sb,
                             rhs=g_t[:, : sbn * DB], start=True, stop=True)
            yg = y_grp[si // GRP]
            yo = (si % GRP) * SBLK * DB
            nc.scalar.activation(yg[:, yo: yo + sbn * DB], ps2[:, : sbn * DB],
                                 LRELU, bias=b1_sb, scale=1.0, alpha=SLOPE)
            if si % GRP == GRP - 1 or si == NSB - 1:
                gi = si // GRP
                nc.scalar.dma_start(yout[:, gd0[gi]:gd0[gi + 1]], yg[:])

        for si in range(NSB + 1):
            if si < NSB:
                phase_a(si)
            if si >= 1:
                phase_b(si - 1)


def declare_tensors(nc, totcols):
    d = nc.dram_tensor
    ins = dict(
        msgs=d("msgs", [128, totcols, H], F8, kind="ExternalInput")[:, :, :],
        xt=d("xt", [H, PN_PAD], F16, kind="ExternalInput")[:, :],
        wblob=d("wblob", [H, 2 * H + OUT_C + 3], F16,
                kind="ExternalInput")[:, :],
        bblob=d("bblob", [H, 3], F32, kind="ExternalInput")[:, :],
        sconst=d("sconst", [128, DB], F8, kind="ExternalInput")[:, :],
    )
    outs = dict(y=d("y", [OUT_C, PN_PAD], F16, kind="ExternalOutput")[:, :])
    return ins, outs


def build_nc(ncols, col_off):
    nc = bacc.Bacc("TRN2", target_bir_lowering=False, debug=False,
                   num_devices=N_CORES)
    ins, outs = declare_tensors(nc, int(col_off[-1]))
    with tile.TileContext(nc) as tc:
        build_kernel_body(tc, ncols, col_off, outs, ins)
    nc.compile()
    return nc


def make_in_maps(inputs):
    hw = host_weights(inputs)
    edge_index = np.asarray(inputs["edge_index"])
    src = edge_index[0].astype(np.int64)
    dst = edge_index[1].astype(np.int64)
    w = np.asarray(inputs["edge_weight"], np.float32)
    x = np.asarray(inputs["x"], np.float32)

    ncols, col_off, node_r = plan(dst)

    bias = np.zeros((H, 3), np.float16)
    wblob = np.concatenate([hw["NW2"], hw["W0T"], hw["W1T"], bias], axis=1)
    bblob = np.zeros((H, 3), np.float32)
    bblob[:, 0:1] = hw["cvec"]
    bblob[:, 1:2] = hw["b0"]
    bblob[:OUT_C, 2:3] = hw["b1"]
    sconst = make_sconst()

    # node n -> (core, column b*64 + pos) under the degree-sorted permutation
    b_n = node_r // 512
    core_n = (node_r % 512) // DB
    pos_n = node_r % DB
    colpos = b_n * DB + pos_n

    in_maps = []
    for k in range(N_CORES):
        msg = host_prep_core(k, src, dst, w, hw["T"], node_r, col_off)
        xtk = np.zeros((H, PN_PAD), np.float16)
        mk = core_n == k
        xtk[:, colpos[mk]] = x[mk].T
        in_maps.append(dict(
            msgs=msg, xt=np.ascontiguousarray(xtk),
            wblob=np.ascontiguousarray(wblob), bblob=bblob, sconst=sconst,
        ))
    return in_maps, ncols, col_off, (core_n, colpos)


_CACHE = {}
LAST_RESULTS = None


def kernel(**inputs) -> np.ndarray:
    global LAST_RESULTS
    import os
    from concourse.bass_utils import run_bass_kernel_spmd

    in_maps, ncols, col_off, (core_n, colpos) = make_in_maps(inputs)

    key = ("nc", tuple(int(v) for v in ncols))
    if key not in _CACHE:
        _CACHE[key] = build_nc(ncols, col_off)
    nc = _CACHE[key]

    trace = bool(int(os.environ.get("LINKX_TRACE", "0")))
    res = run_bass_kernel_spmd(nc, in_maps, core_ids=list(range(N_CORES)),
                               trace=trace)
    LAST_RESULTS = res
    out = np.empty((N_NODES, OUT_C), np.float32)
    for k in range(N_CORES):
        yk = np.asarray(res.results[k]["y"], np.float32)
        mk = core_n == k
        out[mk] = yk[:, colpos[mk]].T
    return out
